# revision 54
# baseline (speedup 1.0000x reference)
"""Trainium2 Bass kernel for nn_AttentionLayer (B=4, S=2048, H=16, DH=64).

Sharding: 8 cores = 4 batches x 2 head-halves. Core c handles batch c//2,
heads (c%2)*8 .. (c%2)*8+8 (512 of the 1024 QKV columns).

Per-core structure (ACT-bound design; modeled ~289.3us, ACT busy ~257.6us):
  - All inputs arrive as bf16, host-prearranged so every DMA is contiguous
    per partition (strided gathers pay ~2x on the serial DMA device).
  - Q/K projections (PE, bf16, per head-pair column chunk) write q/k
    transposed via the DVE bias-add: qt/kt[m] [128p = 2 heads x 64 dh, S].
    The first pair-0 unit interleaves k and q matmuls per x-descriptor so
    exp0 fires ~11us in. V projection -> vt[kb] [128 kpos, 8 heads, 65]
    bf16 with col 64 = 1.0 (PV accumulates the softmax denominator free).
  - Attention stream: 512 slots (group = (head, 512-q block), h-major order
    so consecutive h0/h1 reuse pair-0 tiles and each pair's ~60us window
    hides the next pair's projections). Slots pack into alternating
    3-slot/2-slot PSUM score tiles ([2,2] prefix); one exp (ACT) per tile
    (1536/1024 wide) is the metronome. fp8 DoubleRow scores measured 2%
    rel err (over the 2e-2 gate) -- bf16 q,k (~0.3%) is used instead.
  - PV reoriented to ctx[q, d]: out [128 q, 65] per (slot, qtile), bf16
    E x V at 1.0 cycles/row, accumulated over kb into one memset-zeroed
    PSUM bank per group (start=False + skip_group_check lets 4 sub-bank
    accumulation groups share the bank; a start=True would zero the whole
    2KB zero-region).
  - Normalization: DVE reciprocal of ctx col 64 + ONE stride-0-broadcast
    tensor_mul (rr.broadcast_to), then a single DMA per group.
  - A ~2us stream of throwaway fp32 matmuls at t=0 ramps the PE p-state
    (full clock needs ~3us of continuous execution) during the DMA phase.
  PSUM banks: qkv 2 + scores 3+2 + ctx 1 = 8. Emission order defines Tile
  dependencies, so all projections are emitted (low-priority fill band)
  before the attention stream; band priorities interleave them at schedule
  time. Modeled: ACT 257.6us busy, PE ~249us busy, gaps ~29us
  (9 startup DMA-bound + 5 tail + scattered PE-saturation transients).
"""

import numpy as np

B, S, H, DH = 4, 2048, 16, 64
D = H * DH  # 1024
NCORES = 8
COLS = 512  # qkv columns per core (8 heads)
NKB = 16
EXP_SCALE = 0.125  # 1/sqrt(DH)

_CACHE = {}


def _build():
    import concourse.mybir as mybir
    import concourse.tile as tile
    from concourse import bacc

    f32 = mybir.dt.float32
    f32r = mybir.dt.float32r
    bf16 = mybir.dt.bfloat16
    Exp = mybir.ActivationFunctionType.Exp

    nc = bacc.Bacc(
        "TRN2",
        target_bir_lowering=False,
        debug=False,
        enable_asserts=False,
        num_devices=NCORES,
    )

    xT_d = nc.dram_tensor("xT", [128, 4, 8, 512], bf16, kind="ExternalInput").ap()
    wq_d = nc.dram_tensor("wq", [128, 4, 8, 128], bf16, kind="ExternalInput").ap()
    wk_d = nc.dram_tensor("wk", [128, 4, 8, 128], bf16, kind="ExternalInput").ap()
    wv_d = nc.dram_tensor("wv", [128, 8, COLS], bf16, kind="ExternalInput").ap()
    bq_d = nc.dram_tensor("bq", [COLS], f32, kind="ExternalInput").ap()
    bk_d = nc.dram_tensor("bk", [COLS], f32, kind="ExternalInput").ap()
    bv_d = nc.dram_tensor("bv", [COLS], f32, kind="ExternalInput").ap()
    out_d = nc.dram_tensor("out", [S, COLS], f32, kind="ExternalOutput").ap()

    with tile.TileContext(nc) as tc:
        with (
            tc.tile_pool(name="consts", bufs=1) as consts,
            tc.tile_pool(name="wpool", bufs=1) as wpool,
            tc.tile_pool(name="qkp", bufs=1) as qkp,
            tc.tile_pool(name="vpool", bufs=1) as vpool,
            tc.tile_pool(name="xpool", bufs=1) as xpool,
            tc.tile_pool(name="epool", bufs=1) as epool,
            tc.tile_pool(name="opool", bufs=1) as opool,
            tc.tile_pool(name="psum", bufs=1, space="PSUM") as psum,
        ):
            from contextlib import contextmanager

            base = tc.cur_priority + 50
            att_cur = [base]
            fill_cur = [base + 8000]

            @contextmanager
            def band(cursor):
                off = tc.cur_priority - cursor[0]
                with tc.high_priority(offset=off):
                    yield
                    cursor[0] = tc.cur_priority

            # ---- constants ----
            with band(att_cur):
                warm = consts.tile([1, 1], f32)
                nc.vector.memset(warm, 0.0)
                nc.scalar.activation(warm, warm, Exp)  # pull ACT table load early
                # PE p-state warm-up: the tensor engine reaches full clock
                # only after ~3us of continuous execution. Run ~4us of
                # throwaway fp32 matmuls during the initial DMA window so the
                # first real projections start at full speed.
                wsrc = consts.tile([128, 128], f32, name="wsrc")
                nc.vector.memset(wsrc, 0.0)
                for _ in range(13):
                    wps = psum.tile([128, 4, 65], f32, tag="ctx", bufs=1, name="wps")
                    nc.tensor.matmul(
                        wps.rearrange("p t d -> p (t d)")[:, 0:64],
                        lhsT=wsrc,
                        rhs=wsrc[:, 0:64],
                        start=True,
                        stop=True,
                    )

            with band(fill_cur):
                bq_t = consts.tile([128, 4], f32)
                bk_t = consts.tile([128, 4], f32)
                bv_s = consts.tile([1, COLS], f32)
                bvb = consts.tile([128, COLS], f32)
                nc.gpsimd.dma_start(out=bq_t, in_=bq_d.rearrange("(m p) -> p m", p=128))
                nc.gpsimd.dma_start(out=bk_t, in_=bk_d.rearrange("(m p) -> p m", p=128))
                nc.gpsimd.dma_start(out=bv_s, in_=bv_d[None, :])
                nc.gpsimd.partition_broadcast(bvb, bv_s)

                vt = [vpool.tile([128, 8, 65], bf16, name=f"vt{i}") for i in range(NKB)]
                for i in range(NKB):
                    nc.vector.memset(vt[i][:, :, 64:65], 1.0)

                wv_t = wpool.tile([128, 8, COLS], bf16, name="wv_t")

                # bf16 q/k transposed tiles per head pair m:
                # [128p = 2 heads x 64 dh, 2048 s]
                qt_b = [qkp.tile([128, S], bf16, name=f"qt{m}") for m in range(4)]
                kt_b = [qkp.tile([128, S], bf16, name=f"kt{m}") for m in range(4)]

            # ---- weight chunk ring (2 bufs per proj; quad1 reuses quad0's) ----
            wcur = {"q": {}, "k": {}}

            def load_w(proj, m, eng):
                w_d = wq_d if proj == "q" else wk_d
                wt = wpool.tile(
                    [128, 8, 128], bf16, tag=f"w{proj}", bufs=2, name=f"w{proj}{m}"
                )
                eng.dma_start(out=wt, in_=w_d[:, m, :, :])
                wcur[proj][m] = wt

            with band(fill_cur):
                # critical-path DMA order (all on the SP/HWDGE path; gpsimd
                # dma_start occupies the Pool engine ~1.1us per descriptor):
                # wk0, x0, wk1, x1, wq0, x2, wq1, x3, wv
                xt = []
                for c in range(4):
                    xc = xpool.tile([128, 8, 512], bf16, name=f"xt{c}")
                    xt.append(xc)

                def load_x(c):
                    for j0 in range(0, 8, 2):
                        nc.sync.dma_start(
                            out=xt[c][:, j0 : j0 + 2, :], in_=xT_d[:, c, j0 : j0 + 2, :]
                        )

                load_w("k", 0, nc.sync)
                load_w("q", 0, nc.sync)
                load_x(0)
                load_x(1)
                load_x(2)
                load_x(3)
                load_w("k", 1, nc.sync)
                load_w("q", 1, nc.sync)
                nc.sync.dma_start(out=wv_t, in_=wv_d)



            def proj_kq_fused(m, c):
                # k and q projections for pair m interleaved per x descriptor,
                # so both finish right after the last x chunk lands
                psk = psum.tile([128, 512], f32, tag="qkv", bufs=2, name="psk")
                psq = psum.tile([128, 512], f32, tag="qkv", bufs=2, name="psq2")
                for j in range(8):
                    nc.tensor.matmul(
                        psk, lhsT=wcur["k"][m][:, j, :], rhs=xt[c][:, j, :],
                        start=(j == 0), stop=(j == 7),
                    )
                    nc.tensor.matmul(
                        psq, lhsT=wcur["q"][m][:, j, :], rhs=xt[c][:, j, :],
                        start=(j == 0), stop=(j == 7),
                    )
                nc.vector.tensor_scalar_add(
                    kt_b[m][:, c * 512 : (c + 1) * 512], psk, bk_t[:, m : m + 1]
                )
                nc.vector.tensor_scalar_add(
                    qt_b[m][:, c * 512 : (c + 1) * 512], psq, bq_t[:, m : m + 1]
                )

            # ---- projection unit emitters (fill band) ----
            def proj_qk(proj, m, c):
                dst = qt_b[m] if proj == "q" else kt_b[m]
                bias_t = bq_t if proj == "q" else bk_t
                w = wcur[proj][m]
                ps = psum.tile([128, 512], f32, tag="qkv", bufs=2, name="psq")
                for j in range(8):
                    nc.tensor.matmul(
                        ps,
                        lhsT=w[:, j, :],
                        rhs=xt[c][:, j, :],
                        start=(j == 0),
                        stop=(j == 7),
                    )
                nc.vector.tensor_scalar_add(
                    dst[:, c * 512 : (c + 1) * 512], ps, bias_t[:, m : m + 1]
                )

            def proj_v(m, c, i):
                # V for head-pair m, s-chunk c, seq subchunk i -> vt[4c+i]
                ps = psum.tile([128, 512], f32, tag="qkv", bufs=2, name="psv")
                for j in range(8):
                    nc.tensor.matmul(
                        ps[:, 0:128],
                        lhsT=xt[c][:, j, i * 128 : (i + 1) * 128],
                        rhs=wv_t[:, j, m * 128 : (m + 1) * 128],
                        start=(j == 0),
                        stop=(j == 7),
                    )
                nc.vector.tensor_add(
                    vt[4 * c + i][:, 2 * m : 2 * m + 2, 0:64],
                    ps[:, 0:128].rearrange("p (h d) -> p h d", h=2),
                    bvb[:, m * 128 : (m + 1) * 128].rearrange("p (h d) -> p h d", h=2),
                )

            # projection emission order: priority mirrors consumption
            # (h-major groups: pair m's k/q before pair m's first head).
            proj_order = []
            for m in range(4):
                if m >= 2:
                    proj_order += [("wl", "k", m), ("wl", "q", m)]
                proj_order += [("kq", m, 0), ("k", m, 1), ("q", m, 1)]
                proj_order += [("k", m, 2), ("q", m, 2), ("k", m, 3), ("q", m, 3)]
                proj_order += [("v", m, c, i) for c in range(4) for i in range(4)]

            def emit_proj_all():
                with band(fill_cur):
                    for u in proj_order:
                        if u[0] == "wl":
                            load_w(u[1], u[2], nc.sync)
                        elif u[0] == "v":
                            proj_v(u[1], u[2], u[3])
                        elif u[0] == "kq":
                            proj_kq_fused(u[1], u[2])
                        else:
                            proj_qk(u[0], u[1], u[2])

            # ---- attention stream ----
            # group order: h-major. Each head's 4 q-blocks run consecutively;
            # h0/h1 share pair-0 k/q tiles, so the 21 units of h1 need no new
            # projections -- that window hides pair-1's projections, etc.
            groups = [(h, qb) for h in range(8) for qb in range(4)]
            slots = [(h, qb, kb) for (h, qb) in groups for kb in range(NKB)]
            units = []
            pos = 0
            ui = 0
            prefix = [2, 2]
            while pos < len(slots):
                if ui < len(prefix):
                    w = prefix[ui]
                else:
                    w = 3 if ui % 2 == 0 else 2
                w = min(w, len(slots) - pos)
                units.append(slots[pos : pos + w])
                pos += w
                ui += 1

            ctx_cur = [None]

            def emit_scores(u):
                unit = units[u]
                full = 3 if u % 2 == 0 else 2
                tag = "scA" if u % 2 == 0 else "scB"
                with band(att_cur):
                    sc = psum.tile([128, full, 512], f32, tag=tag, bufs=1, name="sc")
                    for i, (h, qb, kb) in enumerate(unit):
                        m, p0 = h // 2, 64 * (h % 2)
                        nc.tensor.matmul(
                            sc[:, i, :],
                            lhsT=kt_b[m][p0 : p0 + 64, kb * 128 : (kb + 1) * 128],
                            rhs=qt_b[m][p0 : p0 + 64, qb * 512 : (qb + 1) * 512],
                            start=True,
                            stop=True,
                        )
                return sc

            def emit_exp(u, sc):
                w = len(units[u])
                with band(att_cur):
                    ee = epool.tile([128, 3, 512], bf16, tag="e", bufs=34, name="ee")
                    nc.scalar.activation(
                        ee[:, 0:w, :], sc[:, 0:w, :], Exp, scale=EXP_SCALE
                    )
                return ee

            def emit_pv(u, ee):
                unit = units[u]
                with band(att_cur):
                    for i, (h, qb, kb) in enumerate(unit):
                        if kb == 0:
                            ctx_cur[0] = psum.tile(
                                [128, 4, 65], f32, tag="ctx", bufs=1, name="ctx"
                            )
                            nc.vector.memset(ctx_cur[0], 0.0)
                        ctx = ctx_cur[0]
                        for qt in range(4):
                            nc.tensor.matmul(
                                ctx[:, qt, :],
                                lhsT=ee[:, i, qt * 128 : (qt + 1) * 128],
                                rhs=vt[kb][:, h, :],
                                start=False,
                                stop=False,
                                skip_group_check=True,
                            )
                        if kb == NKB - 1:
                            rr = opool.tile([128, 4, 1], f32, tag="r", bufs=3, name="rr")
                            nc.vector.reciprocal(rr, ctx[:, :, 64:65])
                            ob = opool.tile([128, 4, 64], f32, tag="o", bufs=3, name="ob")
                            nc.vector.tensor_mul(
                                ob, ctx[:, :, 0:64], rr.broadcast_to([128, 4, 64])
                            )
                            nc.sync.dma_start(
                                out=out_d[
                                    qb * 512 : (qb + 1) * 512, h * 64 : (h + 1) * 64
                                ].rearrange("(t p) d -> p t d", p=128),
                                in_=ob,
                            )

            emit_proj_all()
            NU = len(units)
            scs = {0: emit_scores(0), 1: emit_scores(1)}
            for u in range(NU):
                ee = emit_exp(u, scs.pop(u))
                if u + 2 < NU:
                    scs[u + 2] = emit_scores(u + 2)
                emit_pv(u, ee)

    nc.compile()
    return nc


def _get_nc():
    if "nc" not in _CACHE:
        _CACHE["nc"] = _build()
    return _CACHE["nc"]


def _in_maps(x, Wq, bq, Wk, bk, Wv, bv):
    import ml_dtypes

    bf = ml_dtypes.bfloat16
    x = np.asarray(x, np.float32)
    maps = []
    for c in range(NCORES):
        b, hh = c // 2, c % 2
        cs = slice(hh * COLS, (hh + 1) * COLS)
        def warr(W):
            # [1024, 512] -> [128 p, 4 m, 8 j, 128 c]
            a = np.asarray(W, np.float32)[:, cs].astype(bf)
            return np.ascontiguousarray(
                a.reshape(8, 128, 4, 128).transpose(1, 2, 0, 3)
            )

        xTr = x[b].T.astype(bf).reshape(8, 128, 4, 512).transpose(1, 2, 0, 3)
        wvr = np.asarray(Wv, np.float32)[:, cs].astype(bf).reshape(8, 128, 512)
        maps.append(
            {
                "xT": np.ascontiguousarray(xTr),
                "wq": warr(Wq),
                "wk": warr(Wk),
                "wv": np.ascontiguousarray(wvr.transpose(1, 0, 2)),
                "bq": np.ascontiguousarray(np.asarray(bq, np.float32)[cs]),
                "bk": np.ascontiguousarray(np.asarray(bk, np.float32)[cs]),
                "bv": np.ascontiguousarray(np.asarray(bv, np.float32)[cs]),
            }
        )
    return maps


def _run(inputs, trace=False):
    from concourse import bass_utils

    nc = _get_nc()
    res = bass_utils.run_bass_kernel_spmd(
        nc,
        _in_maps(**inputs),
        core_ids=list(range(NCORES)),
        trace=trace,
    )
    out = np.empty((B, S, D), np.float32)
    for c in range(NCORES):
        b, hh = c // 2, c % 2
        out[b, :, hh * COLS : (hh + 1) * COLS] = res.results[c]["out"]
    return out, res


def kernel(**inputs):
    out, _ = _run(inputs, trace=False)
    return out


if __name__ == "__main__":
    _get_nc()
    print("build ok")


# revision 57
# speedup vs baseline: 1.0055x; 1.0055x over previous
"""Trainium2 Bass kernel for nn_AttentionLayer (B=4, S=2048, H=16, DH=64).

Sharding: 8 cores = 4 batches x 2 head-halves. Core c handles batch c//2,
heads (c%2)*8 .. (c%2)*8+8 (512 of the 1024 QKV columns).

Per-core structure (ACT-bound design; modeled ~289.3us, ACT busy ~257.6us):
  - All inputs arrive as bf16, host-prearranged so every DMA is contiguous
    per partition (strided gathers pay ~2x on the serial DMA device).
  - Q/K projections (PE, bf16, per head-pair column chunk) write q/k
    transposed via the DVE bias-add: qt/kt[m] [128p = 2 heads x 64 dh, S].
    The first pair-0 unit interleaves k and q matmuls per x-descriptor so
    exp0 fires ~11us in. V projection -> vt[kb] [128 kpos, 8 heads, 65]
    bf16 with col 64 = 1.0 (PV accumulates the softmax denominator free).
  - Attention stream: 512 slots (group = (head, 512-q block), h-major order
    so consecutive h0/h1 reuse pair-0 tiles and each pair's ~60us window
    hides the next pair's projections). Slots pack into alternating
    3-slot/2-slot PSUM score tiles ([2,2] prefix); one exp (ACT) per tile
    (1536/1024 wide) is the metronome. fp8 DoubleRow scores measured 2%
    rel err (over the 2e-2 gate) -- bf16 q,k (~0.3%) is used instead.
  - PV reoriented to ctx[q, d]: out [128 q, 65] per (slot, qtile), bf16
    E x V at 1.0 cycles/row, accumulated over kb into one memset-zeroed
    PSUM bank per group (start=False + skip_group_check lets 4 sub-bank
    accumulation groups share the bank; a start=True would zero the whole
    2KB zero-region).
  - Normalization: DVE reciprocal of ctx col 64 + ONE stride-0-broadcast
    tensor_mul (rr.broadcast_to), then a single DMA per group.
  - A ~2us stream of throwaway fp32 matmuls at t=0 ramps the PE p-state
    (full clock needs ~3us of continuous execution) during the DMA phase.
  PSUM banks: qkv 2 + scores 3+2 + ctx 1 = 8. Emission order defines Tile
  dependencies, so all projections are emitted (low-priority fill band)
  before the attention stream; band priorities interleave them at schedule
  time. Modeled: ACT 257.6us busy, PE ~249us busy, gaps ~29us
  (9 startup DMA-bound + 5 tail + scattered PE-saturation transients).
"""

import numpy as np

B, S, H, DH = 4, 2048, 16, 64
D = H * DH  # 1024
NCORES = 8
COLS = 512  # qkv columns per core (8 heads)
NKB = 16
EXP_SCALE = 0.125  # 1/sqrt(DH)

_CACHE = {}


def _build():
    import concourse.mybir as mybir
    import concourse.tile as tile
    from concourse import bacc

    f32 = mybir.dt.float32
    f32r = mybir.dt.float32r
    bf16 = mybir.dt.bfloat16
    Exp = mybir.ActivationFunctionType.Exp

    nc = bacc.Bacc(
        "TRN2",
        target_bir_lowering=False,
        debug=False,
        enable_asserts=False,
        num_devices=NCORES,
    )

    xT_d = nc.dram_tensor("xT", [128, 4, 8, 512], bf16, kind="ExternalInput").ap()
    wq_d = nc.dram_tensor("wq", [128, 4, 8, 128], bf16, kind="ExternalInput").ap()
    wk_d = nc.dram_tensor("wk", [128, 4, 8, 128], bf16, kind="ExternalInput").ap()
    wv_d = nc.dram_tensor("wv", [128, 8, COLS], bf16, kind="ExternalInput").ap()
    bq_d = nc.dram_tensor("bq", [COLS], f32, kind="ExternalInput").ap()
    bk_d = nc.dram_tensor("bk", [COLS], f32, kind="ExternalInput").ap()
    bv_d = nc.dram_tensor("bv", [COLS], f32, kind="ExternalInput").ap()
    out_d = nc.dram_tensor("out", [S, COLS], f32, kind="ExternalOutput").ap()

    with tile.TileContext(nc) as tc:
        with (
            tc.tile_pool(name="consts", bufs=1) as consts,
            tc.tile_pool(name="wpool", bufs=1) as wpool,
            tc.tile_pool(name="qkp", bufs=1) as qkp,
            tc.tile_pool(name="vpool", bufs=1) as vpool,
            tc.tile_pool(name="xpool", bufs=1) as xpool,
            tc.tile_pool(name="epool", bufs=1) as epool,
            tc.tile_pool(name="opool", bufs=1) as opool,
            tc.tile_pool(name="psum", bufs=1, space="PSUM") as psum,
        ):
            from contextlib import contextmanager

            base = tc.cur_priority + 50
            att_cur = [base]
            fill_cur = [base + 8000]

            @contextmanager
            def band(cursor):
                off = tc.cur_priority - cursor[0]
                with tc.high_priority(offset=off):
                    yield
                    cursor[0] = tc.cur_priority

            # ---- constants ----
            with band(att_cur):
                warm = consts.tile([1, 1], f32)
                nc.vector.memset(warm, 0.0)
                nc.scalar.activation(warm, warm, Exp)  # pull ACT table load early
                # PE p-state warm-up: the tensor engine reaches full clock
                # only after ~3us of continuous execution. Run ~4us of
                # throwaway fp32 matmuls during the initial DMA window so the
                # first real projections start at full speed.
                wsrc = consts.tile([128, 128], f32, name="wsrc")
                nc.vector.memset(wsrc, 0.0)
                for _ in range(13):
                    wps = psum.tile([128, 4, 65], f32, tag="ctx", bufs=1, name="wps")
                    nc.tensor.matmul(
                        wps.rearrange("p t d -> p (t d)")[:, 0:64],
                        lhsT=wsrc,
                        rhs=wsrc[:, 0:64],
                        start=True,
                        stop=True,
                    )

            with band(fill_cur):
                bq_t = consts.tile([128, 4], f32)
                bk_t = consts.tile([128, 4], f32)
                bv_s = consts.tile([1, COLS], f32)
                bvb = consts.tile([128, COLS], f32)
                nc.gpsimd.dma_start(out=bq_t, in_=bq_d.rearrange("(m p) -> p m", p=128))
                nc.gpsimd.dma_start(out=bk_t, in_=bk_d.rearrange("(m p) -> p m", p=128))
                nc.gpsimd.dma_start(out=bv_s, in_=bv_d[None, :])
                nc.gpsimd.partition_broadcast(bvb, bv_s)

                vt = [vpool.tile([128, 8, 65], bf16, name=f"vt{i}") for i in range(NKB)]
                for i in range(NKB):
                    nc.vector.memset(vt[i][:, :, 64:65], 1.0)

                wv_t = wpool.tile([128, 8, COLS], bf16, name="wv_t")

                # bf16 q/k transposed tiles per head pair m:
                # [128p = 2 heads x 64 dh, 2048 s]
                qt_b = [qkp.tile([128, S], bf16, name=f"qt{m}") for m in range(4)]
                kt_b = [qkp.tile([128, S], bf16, name=f"kt{m}") for m in range(4)]

            # ---- weight chunk ring (2 bufs per proj; quad1 reuses quad0's) ----
            wcur = {"q": {}, "k": {}}

            def load_w(proj, m, eng):
                w_d = wq_d if proj == "q" else wk_d
                wt = wpool.tile(
                    [128, 8, 128], bf16, tag=f"w{proj}", bufs=2, name=f"w{proj}{m}"
                )
                eng.dma_start(out=wt, in_=w_d[:, m, :, :])
                wcur[proj][m] = wt

            with band(fill_cur):
                # critical-path DMA order (all on the SP/HWDGE path; gpsimd
                # dma_start occupies the Pool engine ~1.1us per descriptor):
                # wk0, x0, wk1, x1, wq0, x2, wq1, x3, wv
                xt = []
                for c in range(4):
                    xc = xpool.tile([128, 8, 512], bf16, name=f"xt{c}")
                    xt.append(xc)

                def load_x(c):
                    for j0 in range(0, 8, 2):
                        nc.sync.dma_start(
                            out=xt[c][:, j0 : j0 + 2, :], in_=xT_d[:, c, j0 : j0 + 2, :]
                        )

                load_w("k", 0, nc.sync)
                load_w("q", 0, nc.sync)
                load_x(0)
                load_x(1)
                load_x(2)
                load_x(3)
                load_w("k", 1, nc.sync)
                load_w("q", 1, nc.sync)
                nc.sync.dma_start(out=wv_t, in_=wv_d)



            def proj_kq_fused(m, c):
                # k and q projections for pair m interleaved per x descriptor,
                # so both finish right after the last x chunk lands
                psk = psum.tile([128, 512], f32, tag="qkv", bufs=2, name="psk")
                psq = psum.tile([128, 512], f32, tag="qkv", bufs=2, name="psq2")
                for j in range(8):
                    nc.tensor.matmul(
                        psk, lhsT=wcur["k"][m][:, j, :], rhs=xt[c][:, j, :],
                        start=(j == 0), stop=(j == 7),
                    )
                    nc.tensor.matmul(
                        psq, lhsT=wcur["q"][m][:, j, :], rhs=xt[c][:, j, :],
                        start=(j == 0), stop=(j == 7),
                    )
                nc.vector.tensor_scalar_add(
                    kt_b[m][:, c * 512 : (c + 1) * 512], psk, bk_t[:, m : m + 1]
                )
                nc.vector.tensor_scalar_add(
                    qt_b[m][:, c * 512 : (c + 1) * 512], psq, bq_t[:, m : m + 1]
                )

            # ---- projection unit emitters (fill band) ----
            def proj_qk(proj, m, c):
                dst = qt_b[m] if proj == "q" else kt_b[m]
                bias_t = bq_t if proj == "q" else bk_t
                w = wcur[proj][m]
                ps = psum.tile([128, 512], f32, tag="qkv", bufs=2, name="psq")
                for j in range(8):
                    nc.tensor.matmul(
                        ps,
                        lhsT=w[:, j, :],
                        rhs=xt[c][:, j, :],
                        start=(j == 0),
                        stop=(j == 7),
                    )
                nc.vector.tensor_scalar_add(
                    dst[:, c * 512 : (c + 1) * 512], ps, bias_t[:, m : m + 1]
                )

            def proj_v(m, c, i):
                # V for head-pair m, s-chunk c, seq subchunk i -> vt[4c+i]
                ps = psum.tile([128, 512], f32, tag="qkv", bufs=2, name="psv")
                for j in range(8):
                    nc.tensor.matmul(
                        ps[:, 0:128],
                        lhsT=xt[c][:, j, i * 128 : (i + 1) * 128],
                        rhs=wv_t[:, j, m * 128 : (m + 1) * 128],
                        start=(j == 0),
                        stop=(j == 7),
                    )
                nc.vector.tensor_add(
                    vt[4 * c + i][:, 2 * m : 2 * m + 2, 0:64],
                    ps[:, 0:128].rearrange("p (h d) -> p h d", h=2),
                    bvb[:, m * 128 : (m + 1) * 128].rearrange("p (h d) -> p h d", h=2),
                )

            # projection emission order: priority mirrors consumption
            # (h-major groups: pair m's k/q before pair m's first head).
            proj_order = []
            for m in range(4):
                if m >= 2:
                    proj_order += [("wl", "k", m), ("wl", "q", m)]
                proj_order += [("kq", m, 0), ("k", m, 1), ("q", m, 1)]
                proj_order += [("k", m, 2), ("q", m, 2), ("k", m, 3), ("q", m, 3)]
                proj_order += [("v", m, c, i) for c in range(4) for i in range(4)]

            def emit_proj_all():
                with band(fill_cur):
                    for u in proj_order:
                        if u[0] == "wl":
                            load_w(u[1], u[2], nc.sync)
                        elif u[0] == "v":
                            proj_v(u[1], u[2], u[3])
                        elif u[0] == "kq":
                            proj_kq_fused(u[1], u[2])
                        else:
                            proj_qk(u[0], u[1], u[2])

            # ---- attention stream ----
            # group order: h-major. Each head's 4 q-blocks run consecutively;
            # h0/h1 share pair-0 k/q tiles, so the 21 units of h1 need no new
            # projections -- that window hides pair-1's projections, etc.
            groups = [(h, qb) for h in range(8) for qb in range(4)]
            groups.remove((1, 0))
            # interleave (h0,qb0) and (h1,qb0): they share all pair-0 tiles,
            # so consuming both per k-block matches the serial-DMA delivery
            # rate of x chunks during startup (no ACT stalls). Group (1,0)'s
            # PVs are deferred via the E ring until (0,0)'s norm frees the
            # single ctx bank.
            slots = []
            for kb in range(NKB):
                slots.append((0, 0, kb))
                slots.append((1, 0, kb))
            slots += [(h, qb, kb) for (h, qb) in groups[1:] for kb in range(NKB)]
            units = []
            pos = 0
            ui = 0
            prefix = [2, 2]
            while pos < len(slots):
                if ui < len(prefix):
                    w = prefix[ui]
                else:
                    w = 3 if ui % 2 == 0 else 2
                w = min(w, len(slots) - pos)
                units.append(slots[pos : pos + w])
                pos += w
                ui += 1

            ctx_cur = [None]

            def emit_scores(u):
                unit = units[u]
                full = 3 if u % 2 == 0 else 2
                tag = "scA" if u % 2 == 0 else "scB"
                with band(att_cur):
                    sc = psum.tile([128, full, 512], f32, tag=tag, bufs=1, name="sc")
                    for i, (h, qb, kb) in enumerate(unit):
                        m, p0 = h // 2, 64 * (h % 2)
                        nc.tensor.matmul(
                            sc[:, i, :],
                            lhsT=kt_b[m][p0 : p0 + 64, kb * 128 : (kb + 1) * 128],
                            rhs=qt_b[m][p0 : p0 + 64, qb * 512 : (qb + 1) * 512],
                            start=True,
                            stop=True,
                        )
                return sc

            def emit_exp(u, sc):
                w = len(units[u])
                with band(att_cur):
                    ee = epool.tile([128, 3, 512], bf16, tag="e", bufs=34, name="ee")
                    nc.scalar.activation(
                        ee[:, 0:w, :], sc[:, 0:w, :], Exp, scale=EXP_SCALE
                    )
                return ee

            def do_pv(ee, i, h, qb, kb):
                ctx = ctx_cur[0]
                for qt in range(4):
                    nc.tensor.matmul(
                        ctx[:, qt, :],
                        lhsT=ee[:, i, qt * 128 : (qt + 1) * 128],
                        rhs=vt[kb][:, h, :],
                        start=False,
                        stop=False,
                        skip_group_check=True,
                    )

            def do_norm(h, qb):
                ctx = ctx_cur[0]
                rr = opool.tile([128, 4, 1], f32, tag="r", bufs=3, name="rr")
                nc.vector.reciprocal(rr, ctx[:, :, 64:65])
                ob = opool.tile([128, 4, 64], f32, tag="o", bufs=3, name="ob")
                nc.vector.tensor_mul(
                    ob, ctx[:, :, 0:64], rr.broadcast_to([128, 4, 64])
                )
                nc.sync.dma_start(
                    out=out_d[
                        qb * 512 : (qb + 1) * 512, h * 64 : (h + 1) * 64
                    ].rearrange("(t p) d -> p t d", p=128),
                    in_=ob,
                )

            deferred = []

            def emit_pv(u, ee):
                unit = units[u]
                with band(att_cur):
                    for i, (h, qb, kb) in enumerate(unit):
                        if h == 1 and qb == 0:
                            # group B of the startup interleave: stash; its E
                            # tiles stay alive in the deep ring
                            deferred.append((ee, i, kb))
                            if kb == NKB - 1:
                                ctx_cur[0] = psum.tile(
                                    [128, 4, 65], f32, tag="ctx", bufs=1, name="ctx"
                                )
                                nc.vector.memset(ctx_cur[0], 0.0)
                                for dee, di, dkb in deferred:
                                    do_pv(dee, di, 1, 0, dkb)
                                do_norm(1, 0)
                            continue
                        if kb == 0:
                            ctx_cur[0] = psum.tile(
                                [128, 4, 65], f32, tag="ctx", bufs=1, name="ctx"
                            )
                            nc.vector.memset(ctx_cur[0], 0.0)
                        do_pv(ee, i, h, qb, kb)
                        if kb == NKB - 1:
                            do_norm(h, qb)

            emit_proj_all()
            NU = len(units)
            scs = {0: emit_scores(0), 1: emit_scores(1)}
            for u in range(NU):
                ee = emit_exp(u, scs.pop(u))
                if u + 2 < NU:
                    scs[u + 2] = emit_scores(u + 2)
                emit_pv(u, ee)

    nc.compile()
    return nc


def _get_nc():
    if "nc" not in _CACHE:
        _CACHE["nc"] = _build()
    return _CACHE["nc"]


def _in_maps(x, Wq, bq, Wk, bk, Wv, bv):
    import ml_dtypes

    bf = ml_dtypes.bfloat16
    x = np.asarray(x, np.float32)
    maps = []
    for c in range(NCORES):
        b, hh = c // 2, c % 2
        cs = slice(hh * COLS, (hh + 1) * COLS)
        def warr(W):
            # [1024, 512] -> [128 p, 4 m, 8 j, 128 c]
            a = np.asarray(W, np.float32)[:, cs].astype(bf)
            return np.ascontiguousarray(
                a.reshape(8, 128, 4, 128).transpose(1, 2, 0, 3)
            )

        xTr = x[b].T.astype(bf).reshape(8, 128, 4, 512).transpose(1, 2, 0, 3)
        wvr = np.asarray(Wv, np.float32)[:, cs].astype(bf).reshape(8, 128, 512)
        maps.append(
            {
                "xT": np.ascontiguousarray(xTr),
                "wq": warr(Wq),
                "wk": warr(Wk),
                "wv": np.ascontiguousarray(wvr.transpose(1, 0, 2)),
                "bq": np.ascontiguousarray(np.asarray(bq, np.float32)[cs]),
                "bk": np.ascontiguousarray(np.asarray(bk, np.float32)[cs]),
                "bv": np.ascontiguousarray(np.asarray(bv, np.float32)[cs]),
            }
        )
    return maps


def _run(inputs, trace=False):
    from concourse import bass_utils

    nc = _get_nc()
    res = bass_utils.run_bass_kernel_spmd(
        nc,
        _in_maps(**inputs),
        core_ids=list(range(NCORES)),
        trace=trace,
    )
    out = np.empty((B, S, D), np.float32)
    for c in range(NCORES):
        b, hh = c // 2, c % 2
        out[b, :, hh * COLS : (hh + 1) * COLS] = res.results[c]["out"]
    return out, res


def kernel(**inputs):
    out, _ = _run(inputs, trace=False)
    return out


if __name__ == "__main__":
    _get_nc()
    print("build ok")


# revision 59
# speedup vs baseline: 1.0066x; 1.0011x over previous
"""Trainium2 Bass kernel for nn_AttentionLayer (B=4, S=2048, H=16, DH=64).

Sharding: 8 cores = 4 batches x 2 head-halves. Core c handles batch c//2,
heads (c%2)*8 .. (c%2)*8+8 (512 of the 1024 QKV columns).

Per-core structure (ACT-bound design; modeled ~289.3us, ACT busy ~257.6us):
  - All inputs arrive as bf16, host-prearranged so every DMA is contiguous
    per partition (strided gathers pay ~2x on the serial DMA device).
  - Q/K projections (PE, bf16, per head-pair column chunk) write q/k
    transposed via the DVE bias-add: qt/kt[m] [128p = 2 heads x 64 dh, S].
    The first pair-0 unit interleaves k and q matmuls per x-descriptor so
    exp0 fires ~11us in. V projection -> vt[kb] [128 kpos, 8 heads, 65]
    bf16 with col 64 = 1.0 (PV accumulates the softmax denominator free).
  - Attention stream: 512 slots (group = (head, 512-q block), h-major order
    so consecutive h0/h1 reuse pair-0 tiles and each pair's ~60us window
    hides the next pair's projections). Slots pack into alternating
    3-slot/2-slot PSUM score tiles ([2,2] prefix); one exp (ACT) per tile
    (1536/1024 wide) is the metronome. fp8 DoubleRow scores measured 2%
    rel err (over the 2e-2 gate) -- bf16 q,k (~0.3%) is used instead.
  - PV reoriented to ctx[q, d]: out [128 q, 65] per (slot, qtile), bf16
    E x V at 1.0 cycles/row, accumulated over kb into one memset-zeroed
    PSUM bank per group (start=False + skip_group_check lets 4 sub-bank
    accumulation groups share the bank; a start=True would zero the whole
    2KB zero-region).
  - Normalization: DVE reciprocal of ctx col 64 + ONE stride-0-broadcast
    tensor_mul (rr.broadcast_to), then a single DMA per group.
  - A ~2us stream of throwaway fp32 matmuls at t=0 ramps the PE p-state
    (full clock needs ~3us of continuous execution) during the DMA phase.
  PSUM banks: qkv 2 + scores 3+2 + ctx 1 = 8. Emission order defines Tile
  dependencies, so all projections are emitted (low-priority fill band)
  before the attention stream; band priorities interleave them at schedule
  time. Modeled: ACT 257.6us busy, PE ~249us busy, gaps ~29us
  (9 startup DMA-bound + 5 tail + scattered PE-saturation transients).
"""

import numpy as np

B, S, H, DH = 4, 2048, 16, 64
D = H * DH  # 1024
NCORES = 8
COLS = 512  # qkv columns per core (8 heads)
NKB = 16
EXP_SCALE = 0.125  # 1/sqrt(DH)

_CACHE = {}


def _build():
    import concourse.mybir as mybir
    import concourse.tile as tile
    from concourse import bacc

    f32 = mybir.dt.float32
    f32r = mybir.dt.float32r
    bf16 = mybir.dt.bfloat16
    Exp = mybir.ActivationFunctionType.Exp

    nc = bacc.Bacc(
        "TRN2",
        target_bir_lowering=False,
        debug=False,
        enable_asserts=False,
        num_devices=NCORES,
    )

    xT_d = nc.dram_tensor("xT", [128, 4, 8, 512], bf16, kind="ExternalInput").ap()
    wq_d = nc.dram_tensor("wq", [128, 4, 8, 128], bf16, kind="ExternalInput").ap()
    wk_d = nc.dram_tensor("wk", [128, 4, 8, 128], bf16, kind="ExternalInput").ap()
    wv_d = nc.dram_tensor("wv", [128, 8, COLS], bf16, kind="ExternalInput").ap()
    bqk_d = nc.dram_tensor("bqk", [128, 8], f32, kind="ExternalInput").ap()
    bv_d = nc.dram_tensor("bv", [COLS], f32, kind="ExternalInput").ap()
    out_d = nc.dram_tensor("out", [S, COLS], f32, kind="ExternalOutput").ap()

    with tile.TileContext(nc) as tc:
        with (
            tc.tile_pool(name="consts", bufs=1) as consts,
            tc.tile_pool(name="wpool", bufs=1) as wpool,
            tc.tile_pool(name="qkp", bufs=1) as qkp,
            tc.tile_pool(name="vpool", bufs=1) as vpool,
            tc.tile_pool(name="xpool", bufs=1) as xpool,
            tc.tile_pool(name="epool", bufs=1) as epool,
            tc.tile_pool(name="opool", bufs=1) as opool,
            tc.tile_pool(name="psum", bufs=1, space="PSUM") as psum,
        ):
            from contextlib import contextmanager

            base = tc.cur_priority + 50
            att_cur = [base]
            fill_cur = [base + 8000]

            @contextmanager
            def band(cursor):
                off = tc.cur_priority - cursor[0]
                with tc.high_priority(offset=off):
                    yield
                    cursor[0] = tc.cur_priority

            # ---- constants ----
            with band(att_cur):
                warm = consts.tile([1, 1], f32)
                nc.vector.memset(warm, 0.0)
                nc.scalar.activation(warm, warm, Exp)  # pull ACT table load early
                # PE p-state warm-up: the tensor engine reaches full clock
                # only after ~3us of continuous execution. Run ~4us of
                # throwaway fp32 matmuls during the initial DMA window so the
                # first real projections start at full speed.
                wsrc = consts.tile([128, 128], f32, name="wsrc")
                nc.vector.memset(wsrc, 0.0)
                for _ in range(13):
                    wps = psum.tile([128, 4, 65], f32, tag="ctx", bufs=1, name="wps")
                    nc.tensor.matmul(
                        wps.rearrange("p t d -> p (t d)")[:, 0:64],
                        lhsT=wsrc,
                        rhs=wsrc[:, 0:64],
                        start=True,
                        stop=True,
                    )

            with band(fill_cur):
                bqk_t = consts.tile([128, 8], f32)
                bv_s = consts.tile([1, COLS], f32)
                bvb = consts.tile([128, COLS], f32)
                nc.gpsimd.dma_start(out=bqk_t, in_=bqk_d)
                bq_t = bqk_t[:, 0:4]
                bk_t = bqk_t[:, 4:8]
                nc.gpsimd.dma_start(out=bv_s, in_=bv_d[None, :])
                nc.gpsimd.partition_broadcast(bvb, bv_s)

                vt = [vpool.tile([128, 8, 65], bf16, name=f"vt{i}") for i in range(NKB)]
                for i in range(NKB):
                    nc.vector.memset(vt[i][:, :, 64:65], 1.0)

                wv_t = wpool.tile([128, 8, COLS], bf16, name="wv_t")

                # bf16 q/k transposed tiles per head pair m:
                # [128p = 2 heads x 64 dh, 2048 s]
                qt_b = [qkp.tile([128, S], bf16, name=f"qt{m}") for m in range(4)]
                kt_b = [qkp.tile([128, S], bf16, name=f"kt{m}") for m in range(4)]

            # ---- weight chunk ring (2 bufs per proj; quad1 reuses quad0's) ----
            wcur = {"q": {}, "k": {}}

            def load_w(proj, m, eng):
                w_d = wq_d if proj == "q" else wk_d
                wt = wpool.tile(
                    [128, 8, 128], bf16, tag=f"w{proj}", bufs=2, name=f"w{proj}{m}"
                )
                eng.dma_start(out=wt, in_=w_d[:, m, :, :])
                wcur[proj][m] = wt

            with band(fill_cur):
                # critical-path DMA order (all on the SP/HWDGE path; gpsimd
                # dma_start occupies the Pool engine ~1.1us per descriptor):
                # wk0, x0, wk1, x1, wq0, x2, wq1, x3, wv
                xt = []
                for c in range(4):
                    xc = xpool.tile([128, 8, 512], bf16, name=f"xt{c}")
                    xt.append(xc)

                def load_x(c):
                    for j0 in range(0, 8, 2):
                        nc.sync.dma_start(
                            out=xt[c][:, j0 : j0 + 2, :], in_=xT_d[:, c, j0 : j0 + 2, :]
                        )

                load_w("k", 0, nc.sync)
                load_w("q", 0, nc.sync)
                load_x(0)
                load_x(1)
                load_x(2)
                load_x(3)
                load_w("k", 1, nc.sync)
                load_w("q", 1, nc.sync)
                nc.sync.dma_start(out=wv_t, in_=wv_d)



            def proj_kq_fused(m, c):
                # k and q projections for pair m interleaved per x descriptor,
                # so both finish right after the last x chunk lands
                psk = psum.tile([128, 512], f32, tag="qkv", bufs=2, name="psk")
                psq = psum.tile([128, 512], f32, tag="qkv", bufs=2, name="psq2")
                for j in range(8):
                    nc.tensor.matmul(
                        psk, lhsT=wcur["k"][m][:, j, :], rhs=xt[c][:, j, :],
                        start=(j == 0), stop=(j == 7),
                    )
                    nc.tensor.matmul(
                        psq, lhsT=wcur["q"][m][:, j, :], rhs=xt[c][:, j, :],
                        start=(j == 0), stop=(j == 7),
                    )
                nc.vector.tensor_scalar_add(
                    kt_b[m][:, c * 512 : (c + 1) * 512], psk, bk_t[:, m : m + 1]
                )
                nc.vector.tensor_scalar_add(
                    qt_b[m][:, c * 512 : (c + 1) * 512], psq, bq_t[:, m : m + 1]
                )

            # ---- projection unit emitters (fill band) ----
            def proj_qk(proj, m, c):
                dst = qt_b[m] if proj == "q" else kt_b[m]
                bias_t = bq_t if proj == "q" else bk_t
                w = wcur[proj][m]
                ps = psum.tile([128, 512], f32, tag="qkv", bufs=2, name="psq")
                for j in range(8):
                    nc.tensor.matmul(
                        ps,
                        lhsT=w[:, j, :],
                        rhs=xt[c][:, j, :],
                        start=(j == 0),
                        stop=(j == 7),
                    )
                nc.vector.tensor_scalar_add(
                    dst[:, c * 512 : (c + 1) * 512], ps, bias_t[:, m : m + 1]
                )

            def proj_v(m, c, i):
                # V for head-pair m, s-chunk c, seq subchunk i -> vt[4c+i]
                ps = psum.tile([128, 512], f32, tag="qkv", bufs=2, name="psv")
                for j in range(8):
                    nc.tensor.matmul(
                        ps[:, 0:128],
                        lhsT=xt[c][:, j, i * 128 : (i + 1) * 128],
                        rhs=wv_t[:, j, m * 128 : (m + 1) * 128],
                        start=(j == 0),
                        stop=(j == 7),
                    )
                nc.vector.tensor_add(
                    vt[4 * c + i][:, 2 * m : 2 * m + 2, 0:64],
                    ps[:, 0:128].rearrange("p (h d) -> p h d", h=2),
                    bvb[:, m * 128 : (m + 1) * 128].rearrange("p (h d) -> p h d", h=2),
                )

            # projection emission order: priority mirrors consumption
            # (h-major groups: pair m's k/q before pair m's first head).
            proj_order = []
            for m in range(4):
                if m >= 2:
                    proj_order += [("wl", "k", m), ("wl", "q", m)]
                proj_order += [("kq", m, 0), ("k", m, 1), ("q", m, 1)]
                proj_order += [("k", m, 2), ("q", m, 2), ("k", m, 3), ("q", m, 3)]
                proj_order += [("v", m, c, i) for c in range(4) for i in range(4)]

            def emit_proj_all():
                with band(fill_cur):
                    for u in proj_order:
                        if u[0] == "wl":
                            load_w(u[1], u[2], nc.sync)
                        elif u[0] == "v":
                            proj_v(u[1], u[2], u[3])
                        elif u[0] == "kq":
                            proj_kq_fused(u[1], u[2])
                        else:
                            proj_qk(u[0], u[1], u[2])

            # ---- attention stream ----
            # group order: h-major. Each head's 4 q-blocks run consecutively;
            # h0/h1 share pair-0 k/q tiles, so the 21 units of h1 need no new
            # projections -- that window hides pair-1's projections, etc.
            groups = [(h, qb) for h in range(8) for qb in range(4)]
            groups.remove((1, 0))
            # interleave (h0,qb0) and (h1,qb0): they share all pair-0 tiles,
            # so consuming both per k-block matches the serial-DMA delivery
            # rate of x chunks during startup (no ACT stalls). Group (1,0)'s
            # PVs are deferred via the E ring until (0,0)'s norm frees the
            # single ctx bank.
            slots = []
            for kb in range(NKB):
                slots.append((0, 0, kb))
                slots.append((1, 0, kb))
            slots += [(h, qb, kb) for (h, qb) in groups[1:] for kb in range(NKB)]
            units = []
            pos = 0
            ui = 0
            prefix = [2, 2]
            while pos < len(slots):
                if ui < len(prefix):
                    w = prefix[ui]
                else:
                    w = 3 if ui % 2 == 0 else 2
                w = min(w, len(slots) - pos)
                units.append(slots[pos : pos + w])
                pos += w
                ui += 1

            ctx_cur = [None]

            def emit_scores(u):
                unit = units[u]
                full = 3 if u % 2 == 0 else 2
                tag = "scA" if u % 2 == 0 else "scB"
                with band(att_cur):
                    sc = psum.tile([128, full, 512], f32, tag=tag, bufs=1, name="sc")
                    for i, (h, qb, kb) in enumerate(unit):
                        m, p0 = h // 2, 64 * (h % 2)
                        nc.tensor.matmul(
                            sc[:, i, :],
                            lhsT=kt_b[m][p0 : p0 + 64, kb * 128 : (kb + 1) * 128],
                            rhs=qt_b[m][p0 : p0 + 64, qb * 512 : (qb + 1) * 512],
                            start=True,
                            stop=True,
                        )
                return sc

            def emit_exp(u, sc):
                w = len(units[u])
                with band(att_cur):
                    ee = epool.tile([128, 3, 512], bf16, tag="e", bufs=34, name="ee")
                    nc.scalar.activation(
                        ee[:, 0:w, :], sc[:, 0:w, :], Exp, scale=EXP_SCALE
                    )
                return ee

            def do_pv(ee, i, h, qb, kb):
                ctx = ctx_cur[0]
                for qt in range(4):
                    nc.tensor.matmul(
                        ctx[:, qt, :],
                        lhsT=ee[:, i, qt * 128 : (qt + 1) * 128],
                        rhs=vt[kb][:, h, :],
                        start=False,
                        stop=False,
                        skip_group_check=True,
                    )

            def do_norm(h, qb):
                ctx = ctx_cur[0]
                rr = opool.tile([128, 4, 1], f32, tag="r", bufs=3, name="rr")
                nc.vector.reciprocal(rr, ctx[:, :, 64:65])
                ob = opool.tile([128, 4, 64], f32, tag="o", bufs=3, name="ob")
                nc.vector.tensor_mul(
                    ob, ctx[:, :, 0:64], rr.broadcast_to([128, 4, 64])
                )
                nc.sync.dma_start(
                    out=out_d[
                        qb * 512 : (qb + 1) * 512, h * 64 : (h + 1) * 64
                    ].rearrange("(t p) d -> p t d", p=128),
                    in_=ob,
                )

            deferred = []

            def emit_pv(u, ee):
                unit = units[u]
                with band(att_cur):
                    for i, (h, qb, kb) in enumerate(unit):
                        if h == 1 and qb == 0:
                            # group B of the startup interleave: stash; its E
                            # tiles stay alive in the deep ring
                            deferred.append((ee, i, kb))
                            if kb == NKB - 1:
                                ctx_cur[0] = psum.tile(
                                    [128, 4, 65], f32, tag="ctx", bufs=1, name="ctx"
                                )
                                nc.vector.memset(ctx_cur[0], 0.0)
                                for dee, di, dkb in deferred:
                                    do_pv(dee, di, 1, 0, dkb)
                                do_norm(1, 0)
                            continue
                        if kb == 0:
                            ctx_cur[0] = psum.tile(
                                [128, 4, 65], f32, tag="ctx", bufs=1, name="ctx"
                            )
                            nc.vector.memset(ctx_cur[0], 0.0)
                        do_pv(ee, i, h, qb, kb)
                        if kb == NKB - 1:
                            do_norm(h, qb)

            emit_proj_all()
            NU = len(units)
            scs = {0: emit_scores(0), 1: emit_scores(1)}
            for u in range(NU):
                ee = emit_exp(u, scs.pop(u))
                if u + 2 < NU:
                    scs[u + 2] = emit_scores(u + 2)
                emit_pv(u, ee)

    nc.compile()
    return nc


def _get_nc():
    if "nc" not in _CACHE:
        _CACHE["nc"] = _build()
    return _CACHE["nc"]


def _in_maps(x, Wq, bq, Wk, bk, Wv, bv):
    import ml_dtypes

    bf = ml_dtypes.bfloat16
    x = np.asarray(x, np.float32)
    maps = []
    for c in range(NCORES):
        b, hh = c // 2, c % 2
        cs = slice(hh * COLS, (hh + 1) * COLS)
        def warr(W):
            # [1024, 512] -> [128 p, 4 m, 8 j, 128 c]
            a = np.asarray(W, np.float32)[:, cs].astype(bf)
            return np.ascontiguousarray(
                a.reshape(8, 128, 4, 128).transpose(1, 2, 0, 3)
            )

        xTr = x[b].T.astype(bf).reshape(8, 128, 4, 512).transpose(1, 2, 0, 3)
        wvr = np.asarray(Wv, np.float32)[:, cs].astype(bf).reshape(8, 128, 512)
        maps.append(
            {
                "xT": np.ascontiguousarray(xTr),
                "wq": warr(Wq),
                "wk": warr(Wk),
                "wv": np.ascontiguousarray(wvr.transpose(1, 0, 2)),
                "bqk": np.ascontiguousarray(
                    np.concatenate(
                        [
                            np.asarray(bq, np.float32)[cs].reshape(4, 128).T,
                            np.asarray(bk, np.float32)[cs].reshape(4, 128).T,
                        ],
                        axis=1,
                    )
                ),
                "bv": np.ascontiguousarray(np.asarray(bv, np.float32)[cs]),
            }
        )
    return maps


def _run(inputs, trace=False):
    from concourse import bass_utils

    nc = _get_nc()
    res = bass_utils.run_bass_kernel_spmd(
        nc,
        _in_maps(**inputs),
        core_ids=list(range(NCORES)),
        trace=trace,
    )
    out = np.empty((B, S, D), np.float32)
    for c in range(NCORES):
        b, hh = c // 2, c % 2
        out[b, :, hh * COLS : (hh + 1) * COLS] = res.results[c]["out"]
    return out, res


def kernel(**inputs):
    out, _ = _run(inputs, trace=False)
    return out


if __name__ == "__main__":
    _get_nc()
    print("build ok")
